# revision 1
# baseline (speedup 1.0000x reference)
"""Trainium2 Bass kernel for nn_GAT_81209241633571 (2-layer GAT, 4 heads).

Strategy (8 NeuronCores, SPMD):
  - Edges (plus self-loops) are sorted by destination and sharded by
    destination-node range: core c owns 49 tiles of 128 nodes (50176 padded
    nodes total = 8*49*128). All edges into a core's nodes are processed by
    that core, so segment-softmax and aggregation need no cross-core
    reduction.
  - Layer-1 node tables (xs = x@W + per-node attention dots) are built
    replicated on every core (bf16), written to per-core HBM, then edge
    blocks of 128 gather source rows with the custom dma_gather instruction.
    Per-edge attention source values ride in the same gathered row.
  - Scatter (segment sum) is a one-hot matmul: host-built fp8 one-hot
    matrices (edge->dst-slot) as the PE stationary operand accumulate both
    the weighted feature sums and the softmax denominators in PSUM.
    Destination-side values are expanded edge-wise with the transposed
    one-hot as stationary.
  - Softmax uses exp without max subtraction (logits are O(10), safe in
    f32), and the division by the denominator happens per destination node
    after aggregation (alpha sums to 1, so biases fold linearly).
  - Between layers, each core's slice of the layer-2 node table is
    AllGathered so gathers can read any source row locally.
  - int16 gather indices only span 32768 rows, so edges are grouped into
    blocks whose sources are all < 32768 ("lo") or >= 32768 ("hi"); hi
    blocks gather from a base offset of 32768 rows.

The schedule (block counts per tile) is derived from the runtime edge data
and made uniform across cores by padding, so one NEFF serves all 8 cores.
"""
import os
import sys
import numpy as np
import ml_dtypes

sys.path.insert(0, "/opt/trn_rl_repo")

import concourse.bass as bass
import concourse.bacc as bacc
import concourse.mybir as mybir
from concourse.tile import TileContext
from concourse.bass_utils import run_bass_kernel_spmd
from concourse.library_config import mlp

bf = ml_dtypes.bfloat16
f8 = ml_dtypes.float8_e4m3

N = 50000
E = 800000
F = 128
H = 4
C = 64
NEG = 0.2
ALPHA = 0.2
P = 128
NCORES = 8
HI = 32768
SENT = 255
RUNCAP = 8       # max blocks per dma_gather call (>1024 idx crashes HW Q7)
OCH = 16         # one-hot stream chunk (blocks per DMA)

dt = mybir.dt
Alu = mybir.AluOpType
Act = mybir.ActivationFunctionType


# ---------------------------------------------------------------------------
# Host preprocessing
# ---------------------------------------------------------------------------

def preprocess(edge, n=N, ncores=NCORES):
    """Sort/shard/pad edges; build gather-index and one-hot streams.

    Returns cfg dict with the static schedule and per-core input arrays.
    """
    npad = -(-n * 1 // (P * ncores)) * (P * ncores)
    while npad < n:
        npad += P * ncores
    nt_all = npad // P
    nt_core = nt_all // ncores

    e0 = np.asarray(edge[0], np.int64)
    e1 = np.asarray(edge[1], np.int64)
    loop = np.arange(n, dtype=np.int64)
    src = np.concatenate([e0, loop])
    dst = np.concatenate([e1, loop])
    order = np.argsort(dst, kind="stable")
    src, dst = src[order], dst[order]
    tile_of = dst // P
    core_of = tile_of // nt_core

    # bucket per (core, local tile, half)
    buckets = {}
    for c in range(ncores):
        m = core_of == c
        s_c, d_c, t_c = src[m], dst[m], tile_of[m]
        for j in range(nt_core):
            mm = t_c == c * nt_core + j
            s_t, d_t = s_c[mm], d_c[mm]
            lo = s_t < HI
            buckets[(c, j, 0)] = (s_t[lo], d_t[lo])
            buckets[(c, j, 1)] = (s_t[~lo], d_t[~lo])

    B = np.zeros((nt_core, 2), np.int64)
    for j in range(nt_core):
        for hf in (0, 1):
            mx = max(len(buckets[(c, j, hf)][0]) for c in range(ncores))
            B[j, hf] = -(-mx // P)

    # block stream: per tile, halves ordered by parity for gather-run merging
    halves_of = [(0, 1) if j % 2 == 0 else (1, 0) for j in range(nt_core)]
    tob, bhalf = [], []
    tile_first, tile_last = {}, {}
    for j in range(nt_core):
        nb_t = int(B[j, 0] + B[j, 1])
        if nb_t == 0:
            continue
        tile_first[j] = len(tob)
        for hf in halves_of[j]:
            for _ in range(int(B[j, hf])):
                tob.append(j)
                bhalf.append(hf)
        tile_last[j] = len(tob) - 1
    NB = len(tob)
    tob = np.array(tob, np.int64)
    bhalf = np.array(bhalf, np.int64)

    # gather runs: maximal same-half block runs, capped
    runs = []
    b = 0
    while b < NB:
        e_ = b
        while e_ < NB and bhalf[e_] == bhalf[b] and e_ - b < RUNCAP:
            e_ += 1
        runs.append((b, e_, int(bhalf[b])))
        b = e_

    # per-core streams
    gidx = np.zeros((ncores, NB * P), np.int16)
    dloc = np.full((ncores, NB * P), SENT, np.uint8)
    for c in range(ncores):
        pos = {}
        b = 0
        for j in range(nt_core):
            for hf in halves_of[j]:
                if B[j, hf]:
                    pos[(j, hf)] = b
                    b += int(B[j, hf])
        for j in range(nt_core):
            for hf in (0, 1):
                if not B[j, hf]:
                    continue
                s_t, d_t = buckets[(c, j, hf)]
                k = len(s_t)
                base = pos[(j, hf)] * P
                gidx[c, base:base + k] = (s_t % HI).astype(np.int16)
                tbase = (c * nt_core + j) * P
                dloc[c, base:base + k] = (d_t - tbase).astype(np.uint8)

    # idx layout for dma_gather: [128, NB*128/16] i16, idx i at (g*16 + i%16,
    # i//16) for all 8 groups g
    gidx_t = np.zeros((ncores, 128, NB * P // 16), np.int16)
    for c in range(ncores):
        w = gidx[c].reshape(-1, 16).T    # [16, NB*8]
        gidx_t[c] = np.tile(w, (8, 1))

    # one-hot streams fp8: O[e, b*128+d], OT[d, b*128+e]
    O8 = np.zeros((ncores, 128, NB * P), f8)
    OT8 = np.zeros((ncores, 128, NB * P), f8)
    ar = np.arange(P)
    for c in range(ncores):
        dl = dloc[c].reshape(NB, P)
        oh = (dl[:, :, None] == ar[None, None, :])    # [NB, e, d]
        O8[c] = np.ascontiguousarray(
            oh.transpose(1, 0, 2).reshape(P, NB * P)).astype(f8)
        OT8[c] = np.ascontiguousarray(
            oh.transpose(2, 0, 1).reshape(P, NB * P)).astype(f8)

    return dict(n=n, npad=npad, nt_all=nt_all, nt_core=nt_core,
                ncores=ncores, NB=NB, runs=runs, tob=tob,
                tile_first=tile_first, tile_last=tile_last,
                gidx_t=gidx_t, O8=O8, OT8=OT8)


# ---------------------------------------------------------------------------
# Device program
# ---------------------------------------------------------------------------

def build(cfg, has_bias, has_bias2, phases=5):
    npad, nt_all, nt_core = cfg["npad"], cfg["nt_all"], cfg["nt_core"]
    ncores, NB = cfg["ncores"], cfg["NB"]
    runs, tob = cfg["runs"], cfg["tob"]
    tile_first, tile_last = cfg["tile_first"], cfg["tile_last"]

    nc = bacc.Bacc("TRN2", num_devices=ncores, enable_partition_id=True)
    rg = [list(range(ncores))]

    # inputs
    xT_d = nc.dram_tensor("xT", [nt_all, P, H * F], dt.bfloat16, kind="ExternalInput")
    W_d = nc.dram_tensor("Wb", [H, F, F], dt.bfloat16, kind="ExternalInput")
    WT_d = nc.dram_tensor("WTb", [H, F, F], dt.bfloat16, kind="ExternalInput")
    ac_d = nc.dram_tensor("acol", [H, F, 2], dt.bfloat16, kind="ExternalInput")
    W2_d = nc.dram_tensor("W2b", [4, F, C], dt.bfloat16, kind="ExternalInput")
    W2T_d = nc.dram_tensor("W2Tb", [4, C, F], dt.bfloat16, kind="ExternalInput")
    a2_d = nc.dram_tensor("a2col", [C, 2], dt.bfloat16, kind="ExternalInput")
    gi_d = nc.dram_tensor("gidx", [P, NB * P // 16], dt.int16, kind="ExternalInput")
    O8_d = nc.dram_tensor("O8", [P, NB * P], dt.float8e4, kind="ExternalInput")
    OT8_d = nc.dram_tensor("OT8", [P, NB * P], dt.float8e4, kind="ExternalInput")
    b1_d = nc.dram_tensor("b1rep", [H, P, F], dt.float32, kind="ExternalInput")
    b2_d = nc.dram_tensor("b2rep", [P, C], dt.float32, kind="ExternalInput")
    out_d = nc.dram_tensor("out", [nt_core * P, C], dt.float32, kind="ExternalOutput")

    # internal DRAM
    XTAB = nc.dram_tensor("XTAB", [npad, H * F + 128], dt.bfloat16, kind="Internal")
    ADT = nc.dram_tensor("ADT", [npad, 4], dt.bfloat16, kind="Internal")
    XL1 = nc.dram_tensor("XL1", [nt_core * P, H * F], dt.bfloat16, kind="Internal")
    X2SH = nc.dram_tensor("X2SH", [nt_core * P, 128], dt.bfloat16,
                          kind="Internal")
    A2SH = nc.dram_tensor("A2SH", [nt_core * P, 1], dt.bfloat16,
                          kind="Internal")
    X2G = nc.dram_tensor("X2G", [npad, 128], dt.bfloat16,
                         kind="Internal", addr_space="Shared")
    A2G = nc.dram_tensor("A2G", [npad, 1], dt.bfloat16,
                         kind="Internal", addr_space="Shared")

    EW = H * F + 128          # XTAB row elems (640): 512 xs + 8(as f32) + pad

    with TileContext(nc) as tc:
        nc.gpsimd.load_library(mlp)

        # ---------------- phase 0: weights prep ----------------
        with tc.tile_pool(name="wsb", bufs=1) as wsb, \
             tc.tile_pool(name="wps", bufs=1, space="PSUM") as wps:
            wrhs = []
            for h in range(H):
                wt = wsb.tile([F, F], dt.bfloat16, tag=f"wt{h}")
                nc.sync.dma_start(out=wt[:], in_=WT_d[h])
                acs = wsb.tile([F, 2], dt.bfloat16, tag=f"ac{h}")
                nc.sync.dma_start(out=acs[:], in_=ac_d[h])
                pw = wps.tile([F, 2], dt.float32, tag="pw")
                nc.tensor.matmul(out=pw[:], lhsT=wt[:], rhs=acs[:],
                                 start=True, stop=True)
                wr = wsb.tile([F, F + 2], dt.bfloat16, tag=f"wr{h}")
                nc.sync.dma_start(out=wr[:, 0:F], in_=W_d[h])
                nc.vector.tensor_copy(out=wr[:, F:F + 2], in_=pw[:])
                wrhs.append(wr)
            w2rhs = []
            for k in range(4):
                wt2 = wsb.tile([C, F], dt.bfloat16, tag="wt2")
                nc.sync.dma_start(out=wt2[:], in_=W2T_d[k])
                ac2 = wsb.tile([C, 2], dt.bfloat16, tag="ac2")
                nc.sync.dma_start(out=ac2[:], in_=a2_d[:])
                pw2 = wps.tile([F, 2], dt.float32, tag="pw")
                nc.tensor.matmul(out=pw2[:], lhsT=wt2[:], rhs=ac2[:],
                                 start=True, stop=True)
                w2 = wsb.tile([F, C + 2], dt.bfloat16, tag=f"w2r{k}")
                nc.sync.dma_start(out=w2[:, 0:C], in_=W2_d[k])
                nc.vector.tensor_copy(out=w2[:, C:C + 2], in_=pw2[:])
                w2rhs.append(w2)
            if has_bias:
                b1s = []
                for h in range(H):
                    t = wsb.tile([P, F], dt.float32, tag=f"b1_{h}")
                    nc.sync.dma_start(out=t[:], in_=b1_d[h])
                    b1s.append(t)
            if has_bias2:
                b2s = wsb.tile([P, C], dt.float32, tag="b2")
                nc.sync.dma_start(out=b2s[:], in_=b2_d[:])

            # gather indices resident
            gidx_sb = wsb.tile([P, NB * P // 16], dt.int16, tag="gi")
            nc.sync.dma_start(out=gidx_sb[:], in_=gi_d[:])

            # always write out once so the output is defined even when
            # later phases are disabled
            zo = wsb.tile([P, C], dt.float32, tag="zo")
            nc.gpsimd.memset(zo[:], 0)
            nc.sync.dma_start(out=out_d[0:P, :], in_=zo[:])

            # ---------------- phase 1: L1 tables (replicated) ----------------
            with tc.tile_pool(name="t1", bufs=3) as t1, \
                 tc.tile_pool(name="p1", bufs=4, space="PSUM") as p1:
                for t in range(nt_all if phases >= 1 else 0):
                    xt = t1.tile([P, H * F], dt.bfloat16, tag="xt")
                    nc.sync.dma_start(out=xt[:], in_=xT_d[t])
                    xrow = t1.tile([P, EW], dt.bfloat16, tag="xrow")
                    nc.gpsimd.memset(xrow[:, H * F + 8:EW], 0)
                    adrow = t1.tile([P, 4], dt.bfloat16, tag="adrow")
                    asv = xrow[:, H * F:H * F + 8].bitcast(dt.float32)
                    for h in range(H):
                        ph = p1.tile([P, F + 2], dt.float32, tag="ph")
                        nc.tensor.matmul(out=ph[:], lhsT=xt[:, h * F:(h + 1) * F],
                                         rhs=wrhs[h][:], start=True, stop=True)
                        if has_bias:
                            nc.vector.tensor_tensor(
                                out=xrow[:, h * F:(h + 1) * F], in0=ph[:, 0:F],
                                in1=b1s[h][:], op=Alu.add)
                        elif h % 2 == 0:
                            nc.vector.tensor_copy(
                                out=xrow[:, h * F:(h + 1) * F], in_=ph[:, 0:F])
                        else:
                            nc.scalar.copy(
                                out=xrow[:, h * F:(h + 1) * F], in_=ph[:, 0:F])
                        nc.vector.tensor_copy(out=asv[:, h:h + 1],
                                              in_=ph[:, F:F + 1])
                        nc.vector.tensor_copy(out=adrow[:, h:h + 1],
                                              in_=ph[:, F + 1:F + 2])
                    nc.sync.dma_start(out=XTAB[t * P:(t + 1) * P, :], in_=xrow[:])
                    nc.sync.dma_start(out=ADT[t * P:(t + 1) * P, :], in_=adrow[:])

            # ---------------- phase 2: L1 edge loop ----------------
            # dst-side ad for the core's own dst tiles via pid-dependent DMA
            pid = nc.gpsimd.partition_id()
            off = pid * (nt_core * P)
            adt_loc = wsb.tile([P, nt_core, 4], dt.bfloat16, tag="adtl")
            nc.gpsimd.dma_start(
                out=adt_loc[:],
                in_=ADT[bass.ds(off, nt_core * P)].rearrange(
                    "(t p) c -> p t c", p=P))

            with tc.tile_pool(name="g2", bufs=2) as g2, \
                 tc.tile_pool(name="o2", bufs=2) as o2, \
                 tc.tile_pool(name="s2", bufs=4) as s2, \
                 tc.tile_pool(name="e2", bufs=3) as e2, \
                 tc.tile_pool(name="pp", bufs=2, space="PSUM") as pp, \
                 tc.tile_pool(name="pa", bufs=2, space="PSUM") as pa:
                nch = -(-NB // OCH)
                o_t = [None] * nch
                ot_t = [None] * nch
                ps_out = ps_den = None
                for (b0, b1, hf) in (runs if phases >= 2 else []):
                    nb = b1 - b0
                    xg = g2.tile([P, nb, EW], dt.bfloat16, tag="xg")
                    tab = XTAB[HI:, :] if hf else XTAB[:, :]
                    nc.gpsimd.dma_gather(
                        xg[:], tab, gidx_sb[:, b0 * 8:b1 * 8],
                        nb * P, nb * P, EW)
                    for b in range(b0, b1):
                        ch, coff = b // OCH, b % OCH
                        if o_t[ch] is None:
                            ot = o2.tile([P, OCH * P], dt.float8e4, tag="oc")
                            nc.sync.dma_start(
                                out=ot[:, 0:min(OCH * P, NB * P - ch * OCH * P)],
                                in_=O8_d[:, ch * OCH * P:
                                         min((ch + 1) * OCH * P, NB * P)])
                            ott = o2.tile([P, OCH * P], dt.float8e4, tag="otc")
                            nc.sync.dma_start(
                                out=ott[:, 0:min(OCH * P, NB * P - ch * OCH * P)],
                                in_=OT8_d[:, ch * OCH * P:
                                          min((ch + 1) * OCH * P, NB * P)])
                            o_t[ch], ot_t[ch] = ot, ott
                        j = int(tob[b])
                        Osl = o_t[ch][:, coff * P:(coff + 1) * P]
                        OTsl = ot_t[ch][:, coff * P:(coff + 1) * P]
                        first = b == tile_first[j]
                        last = b == tile_last[j]
                        if first:
                            ps_out = pp.tile([P, H * F], dt.float32, tag="po")
                            ps_den = pa.tile([P, 4], dt.float32, tag="pd")
                        # expand ad -> [e, 4]
                        ps_ad = pa.tile([P, 4], dt.float32, tag="pe")
                        nc.tensor.matmul(out=ps_ad[:], lhsT=OTsl,
                                         rhs=adt_loc[:, j], start=True, stop=True)
                        e4 = e2.tile([P, 4], dt.float32, tag="e4")
                        nc.vector.tensor_tensor(
                            out=e4[:],
                            in0=xg[:, b - b0, H * F:H * F + 8].bitcast(dt.float32),
                            in1=ps_ad[:], op=Alu.add)
                        e4b = e2.tile([P, 4], dt.float32, tag="e4b")
                        nc.vector.tensor_scalar(
                            out=e4b[:], in0=e4[:], scalar1=NEG, scalar2=None,
                            op0=Alu.mult)
                        nc.vector.tensor_tensor(out=e4[:], in0=e4[:], in1=e4b[:],
                                                op=Alu.max)
                        p4f = e2.tile([P, 4], dt.float32, tag="p4f")
                        nc.scalar.activation(p4f[:], e4[:], Act.Exp)
                        p4b = e2.tile([P, 4], dt.bfloat16, tag="p4b")
                        nc.vector.tensor_copy(out=p4b[:], in_=p4f[:])
                        for h in range(H):
                            xp = s2.tile([P, F], dt.bfloat16, tag="xp")
                            xsl = xg[:, b - b0, h * F:(h + 1) * F]
                            if h % 2 == 0:
                                nc.vector.tensor_scalar(
                                    out=xp[:], in0=xsl, scalar1=p4f[:, h:h + 1],
                                    scalar2=None, op0=Alu.mult)
                            else:
                                nc.scalar.activation(
                                    xp[:], xsl, Act.Copy, scale=p4f[:, h:h + 1])
                            nc.tensor.matmul(
                                out=ps_out[:, h * F:(h + 1) * F], lhsT=Osl,
                                rhs=xp[:], start=(first and h == 0),
                                stop=(last and h == H - 1))
                        nc.tensor.matmul(out=ps_den[:], lhsT=Osl, rhs=p4b[:],
                                         start=first, stop=last)
                        if last:
                            dg = e2.tile([P, 4], dt.float32, tag="dg")
                            nc.vector.tensor_scalar(
                                out=dg[:], in0=ps_den[:], scalar1=1e-30,
                                scalar2=None, op0=Alu.max)
                            rc = e2.tile([P, 4], dt.float32, tag="rc")
                            nc.vector.reciprocal(out=rc[:], in_=dg[:])
                            xl1 = s2.tile([P, H * F], dt.bfloat16, tag="xl1")
                            for h in range(H):
                                y = s2.tile([P, F], dt.float32, tag="y")
                                nc.scalar.activation(
                                    y[:], ps_out[:, h * F:(h + 1) * F],
                                    Act.Copy, scale=rc[:, h:h + 1])
                                y2 = s2.tile([P, F], dt.float32, tag="y2")
                                nc.vector.tensor_scalar(
                                    out=y2[:], in0=y[:], scalar1=ALPHA,
                                    scalar2=None, op0=Alu.mult)
                                nc.vector.tensor_tensor(
                                    out=xl1[:, h * F:(h + 1) * F], in0=y[:],
                                    in1=y2[:], op=Alu.max)
                            nc.sync.dma_start(
                                out=XL1[j * P:(j + 1) * P, :], in_=xl1[:])

            # ---------------- phase 3: L2 table (shard) ----------------
            with tc.tile_pool(name="t3", bufs=3) as t3, \
                 tc.tile_pool(name="p3", bufs=2, space="PSUM") as p3:
                for j in range(nt_core if phases >= 3 else 0):
                    ps2 = p3.tile([P, C + 2], dt.float32, tag="ps2")
                    for k in range(4):
                        xt2 = t3.tile([P, F], dt.bfloat16, tag="xt2")
                        nc.sync.dma_start(
                            out=xt2[:],
                            in_=XL1[j * P:(j + 1) * P, k * F:(k + 1) * F],
                            transpose=True)
                        nc.tensor.matmul(out=ps2[:], lhsT=xt2[:],
                                         rhs=w2rhs[k][:], start=(k == 0),
                                         stop=(k == 3))
                    x2row = t3.tile([P, 128], dt.bfloat16, tag="x2row")
                    nc.gpsimd.memset(x2row[:, C + 2:128], 0)
                    if has_bias2:
                        nc.vector.tensor_tensor(out=x2row[:, 0:C],
                                                in0=ps2[:, 0:C], in1=b2s[:],
                                                op=Alu.add)
                    else:
                        nc.vector.tensor_copy(out=x2row[:, 0:C], in_=ps2[:, 0:C])
                    as2v = x2row[:, C:C + 2].bitcast(dt.float32)
                    nc.vector.tensor_copy(out=as2v[:], in_=ps2[:, C:C + 1])
                    ad2 = t3.tile([P, 1], dt.bfloat16, tag="ad2")
                    nc.vector.tensor_copy(out=ad2[:], in_=ps2[:, C + 1:C + 2])
                    nc.sync.dma_start(out=X2SH[j * P:(j + 1) * P, :], in_=x2row[:])
                    nc.sync.dma_start(out=A2SH[j * P:(j + 1) * P, :], in_=ad2[:])

            # ---------------- phase 4: AllGather ----------------
            if phases >= 4:
              nc.gpsimd.collective_compute(
                "AllGather", Alu.bypass, replica_groups=rg,
                ins=[X2SH[:]], outs=[X2G[:]])
              nc.gpsimd.collective_compute(
                "AllGather", Alu.bypass, replica_groups=rg,
                ins=[A2SH[:]], outs=[A2G[:]])
            a2_loc = wsb.tile([P, nt_core, 1], dt.bfloat16, tag="a2l")
            if phases >= 4:
              nc.gpsimd.dma_start(
                out=a2_loc[:],
                in_=A2G[bass.ds(off, nt_core * P)].rearrange(
                    "(t p) c -> p t c", p=P))

            # ---------------- phase 5: L2 edge loop ----------------
            with tc.tile_pool(name="g5", bufs=2) as g5, \
                 tc.tile_pool(name="o5", bufs=2) as o5, \
                 tc.tile_pool(name="s5", bufs=4) as s5, \
                 tc.tile_pool(name="e5", bufs=3) as e5, \
                 tc.tile_pool(name="pq", bufs=2, space="PSUM") as pq, \
                 tc.tile_pool(name="pb", bufs=2, space="PSUM") as pb:
                nch = -(-NB // OCH)
                o_t = [None] * nch
                ot_t = [None] * nch
                ps2o = ps2d = None
                for (b0, b1, hf) in (runs if phases >= 5 else []):
                    nb = b1 - b0
                    xg = g5.tile([P, nb, 128], dt.bfloat16, tag="xg5")
                    tab = X2G[HI:, :] if hf else X2G[:, :]
                    nc.gpsimd.dma_gather(
                        xg[:], tab, gidx_sb[:, b0 * 8:b1 * 8],
                        nb * P, nb * P, 128)
                    for b in range(b0, b1):
                        ch, coff = b // OCH, b % OCH
                        if o_t[ch] is None:
                            ot = o5.tile([P, OCH * P], dt.float8e4, tag="oc5")
                            nc.sync.dma_start(
                                out=ot[:, 0:min(OCH * P, NB * P - ch * OCH * P)],
                                in_=O8_d[:, ch * OCH * P:
                                         min((ch + 1) * OCH * P, NB * P)])
                            ott = o5.tile([P, OCH * P], dt.float8e4, tag="otc5")
                            nc.sync.dma_start(
                                out=ott[:, 0:min(OCH * P, NB * P - ch * OCH * P)],
                                in_=OT8_d[:, ch * OCH * P:
                                          min((ch + 1) * OCH * P, NB * P)])
                            o_t[ch], ot_t[ch] = ot, ott
                        j = int(tob[b])
                        Osl = o_t[ch][:, coff * P:(coff + 1) * P]
                        OTsl = ot_t[ch][:, coff * P:(coff + 1) * P]
                        first = b == tile_first[j]
                        last = b == tile_last[j]
                        if first:
                            ps2o = pq.tile([P, C], dt.float32, tag="p5o")
                            ps2d = pb.tile([P, 1], dt.float32, tag="p5d")
                        ps_ad = pb.tile([P, 1], dt.float32, tag="p5e")
                        nc.tensor.matmul(out=ps_ad[:], lhsT=OTsl,
                                         rhs=a2_loc[:, j], start=True, stop=True)
                        e1t = e5.tile([P, 1], dt.float32, tag="e1")
                        nc.vector.tensor_tensor(
                            out=e1t[:],
                            in0=xg[:, b - b0, C:C + 2].bitcast(dt.float32),
                            in1=ps_ad[:], op=Alu.add)
                        e1b = e5.tile([P, 1], dt.float32, tag="e1b")
                        nc.vector.tensor_scalar(
                            out=e1b[:], in0=e1t[:], scalar1=NEG, scalar2=None,
                            op0=Alu.mult)
                        nc.vector.tensor_tensor(out=e1t[:], in0=e1t[:],
                                                in1=e1b[:], op=Alu.max)
                        p1f = e5.tile([P, 1], dt.float32, tag="p1f")
                        nc.scalar.activation(p1f[:], e1t[:], Act.Exp)
                        p1b = e5.tile([P, 1], dt.bfloat16, tag="p1b")
                        nc.vector.tensor_copy(out=p1b[:], in_=p1f[:])
                        xp = s5.tile([P, C], dt.bfloat16, tag="xp5")
                        nc.vector.tensor_scalar(
                            out=xp[:], in0=xg[:, b - b0, 0:C],
                            scalar1=p1f[:, 0:1], scalar2=None, op0=Alu.mult)
                        nc.tensor.matmul(out=ps2o[:], lhsT=Osl, rhs=xp[:],
                                         start=first, stop=last)
                        nc.tensor.matmul(out=ps2d[:], lhsT=Osl, rhs=p1b[:],
                                         start=first, stop=last)
                        if last:
                            dg = e5.tile([P, 1], dt.float32, tag="dg5")
                            nc.vector.tensor_scalar(
                                out=dg[:], in0=ps2d[:], scalar1=1e-30,
                                scalar2=None, op0=Alu.max)
                            rc = e5.tile([P, 1], dt.float32, tag="rc5")
                            nc.vector.reciprocal(out=rc[:], in_=dg[:])
                            y = s5.tile([P, C], dt.float32, tag="y5")
                            nc.scalar.activation(y[:], ps2o[:], Act.Copy,
                                                 scale=rc[:, 0:1])
                            y2 = s5.tile([P, C], dt.float32, tag="y52")
                            nc.vector.tensor_scalar(
                                out=y2[:], in0=y[:], scalar1=ALPHA,
                                scalar2=None, op0=Alu.mult)
                            nc.vector.tensor_tensor(out=y[:], in0=y[:],
                                                    in1=y2[:], op=Alu.max)
                            yo = s5.tile([P, C], dt.float32, tag="yo")
                            nc.scalar.activation(yo[:], y[:], Act.Tanh)
                            nc.sync.dma_start(
                                out=out_d[j * P:(j + 1) * P, :], in_=yo[:])

    nc.compile()
    return nc


# ---------------------------------------------------------------------------
# Entry point
# ---------------------------------------------------------------------------

_CACHE = {}


def _inputs_for_core(cfg, c, inputs):
    type_emb = np.asarray(inputs["type_emb"], np.float32)
    W = np.asarray(inputs["W"], np.float32)
    a_src = np.asarray(inputs["att_src"], np.float32)
    a_dst = np.asarray(inputs["att_dst"], np.float32)
    W_out = np.asarray(inputs["W_out"], np.float32)
    a2s = np.asarray(inputs["att_src_out"], np.float32)
    a2d = np.asarray(inputs["att_dst_out"], np.float32)
    bias = np.asarray(inputs["bias"], np.float32)
    bias2 = np.asarray(inputs["bias_out"], np.float32)
    npad, nt_all, nt_core = cfg["npad"], cfg["nt_all"], cfg["nt_core"]
    n = cfg["n"]

    # xT tiles: [nt_all, 128fi, H*128n]
    xT = np.zeros((nt_all, P, H * F), bf)
    te = np.zeros((H, npad, F), np.float32)
    te[:, :n] = type_emb
    for t in range(nt_all):
        blk = te[:, t * P:(t + 1) * P, :]          # [H, n128, F]
        xT[t] = np.concatenate([blk[h].T for h in range(H)], axis=1).astype(bf)
    acol = np.stack([np.stack([a_src[h], a_dst[h]], 1) for h in range(H)])
    W2k = np.stack([W_out[k * F:(k + 1) * F] for k in range(4)])
    W2Tk = np.stack([W_out[k * F:(k + 1) * F].T for k in range(4)])
    a2col = np.stack([a2s, a2d], 1)
    return {
        "xT": xT,
        "Wb": W.astype(bf), "WTb": W.transpose(0, 2, 1).astype(bf),
        "acol": acol.astype(bf),
        "W2b": W2k.astype(bf), "W2Tb": W2Tk.astype(bf),
        "a2col": a2col.astype(bf),
        "gidx": cfg["gidx_t"][c], "O8": cfg["O8"][c], "OT8": cfg["OT8"][c],
        "b1rep": np.broadcast_to(bias[:, None, :], (H, P, F)).astype(np.float32).copy(),
        "b2rep": np.broadcast_to(bias2[None, :], (P, C)).astype(np.float32).copy(),
    }


def kernel(**inputs):
    edge = np.asarray(inputs["edge"])
    cfg = preprocess(edge)
    has_bias = bool(np.any(np.asarray(inputs["bias"])))
    has_bias2 = bool(np.any(np.asarray(inputs["bias_out"])))
    key = (cfg["NB"], tuple(cfg["tob"]), has_bias, has_bias2)
    if key not in _CACHE:
        _CACHE[key] = build(cfg, has_bias, has_bias2)
    nc = _CACHE[key]
    in_maps = [_inputs_for_core(cfg, c, inputs) for c in range(NCORES)]
    res = run_bass_kernel_spmd(nc, in_maps, core_ids=list(range(NCORES)))
    outs = [res.results[c]["out"] for c in range(NCORES)]
    full = np.concatenate(outs, 0)[:N]
    return full.astype(np.float32)


if __name__ == "__main__":
    sys.path.insert(0, os.path.dirname(os.path.abspath(__file__)))
    import jax
    with jax.default_device(jax.devices("cpu")[0]):
        import reference
        inputs = {k: np.asarray(v) for k, v in reference.setup_inputs().items()}
        expected = np.asarray(reference.reference(**inputs))
    got = kernel(**inputs)
    rel = np.linalg.norm(got - expected) / np.linalg.norm(expected)
    print("Relative error:", rel)

